# revision 7
# baseline (speedup 1.0000x reference)
"""Trainium2 Bass kernel for nn_DigitalTwinSimulator (2-layer LSTM + AR rollout).

Strategy: pure data parallel across 8 NeuronCores (batch 4096 -> 512/core).
Per core, state is kept feature-on-partitions / batch-on-free-dim.

Encode phase (t = 0..128): the two LSTM layers run in lockstep (layer 1 lags
layer 0 by one step) so all gate matmuls merge into K=128 block matmuls and
all elementwise ops are full 128-partition instructions. Biases + x
contribution enter through a K=5 matmul against [x_t; 1] (ones row built
host-side).

AR phase (steps 128..187): pred feedback is algebraically folded:
  W0x @ pred = (W0x @ Wfc) @ h1 + W0x @ bfc
so the recurrence never materializes pred; the FC head runs off the critical
path purely for output. Layers run sequentially (inherent to AR); biases ride
in a 65th row of the h1 tile (ones row).

Batch is split into 2 chunks of 256 columns that pipeline against each other
to hide the serial dependency chain latency.
"""
import os
import sys

for _p in ("/opt/trn_rl_repo", "/root/.axon_site/_ro/trn_rl_repo"):
    if os.path.isdir(_p) and _p not in sys.path:
        sys.path.append(_p)

import numpy as np

B, T, D, H, STEPS = 4096, 128, 4, 64, 60
NCORES = 8
BC = B // NCORES          # 512 batch rows per core
CH = 2                    # batch chunks per core (pipelined)
CW = BC // CH             # 256 columns per chunk

_cache = {}
TRACE = False
LAST = {}


def _build(T_, STEPS_):
    import concourse.bass as bass
    import concourse.tile as tile
    from concourse import bacc, mybir

    f32 = mybir.dt.float32
    f32r = mybir.dt.float32r
    AF = mybir.ActivationFunctionType
    ALU = mybir.AluOpType

    nc = bacc.Bacc("TRN2", target_bir_lowering=False, debug=False,
                   num_devices=NCORES)

    xt_d = nc.dram_tensor("xt", (T_ + 1, 5, BC), f32, kind="ExternalInput")
    wencblk_d = nc.dram_tensor("wencblk", (128, 512), f32, kind="ExternalInput")
    wencx_d = nc.dram_tensor("wencx", (5, 512), f32, kind="ExternalInput")
    w1h0_d = nc.dram_tensor("w1h0", (64, 256), f32, kind="ExternalInput")
    w1h1b_d = nc.dram_tensor("w1h1b", (65, 256), f32, kind="ExternalInput")
    w0h0_d = nc.dram_tensor("w0h0", (64, 256), f32, kind="ExternalInput")
    w0h1b_d = nc.dram_tensor("w0h1b", (65, 256), f32, kind="ExternalInput")
    wfcb_d = nc.dram_tensor("wfcb", (65, 4), f32, kind="ExternalInput")
    ones_d = nc.dram_tensor("ones_row", (1, BC), f32, kind="ExternalInput")
    out_d = nc.dram_tensor("out", (STEPS_, 4, BC), f32, kind="ExternalOutput")

    with tile.TileContext(nc) as tc:
        with tc.tile_pool(name="const", bufs=1) as cpool, \
             tc.tile_pool(name="state", bufs=1) as spool, \
             tc.tile_pool(name="xin", bufs=4) as xpool, \
             tc.tile_pool(name="act", bufs=3) as apool, \
             tc.tile_pool(name="tmp", bufs=3) as tpool, \
             tc.tile_pool(name="psum", bufs=2, space="PSUM") as ppool:

            def dma_w(shape, src, tag, dt=f32r):
                t = cpool.tile(list(shape), dt, tag=tag)
                nc.sync.dma_start(t[:], src.ap().bitcast(dt) if dt is f32r else src.ap())
                return t

            wencblk = dma_w((128, 512), wencblk_d, "wencblk")
            wencx = dma_w((5, 512), wencx_d, "wencx")
            w1h0 = dma_w((64, 256), w1h0_d, "w1h0")
            w1h1b = dma_w((65, 256), w1h1b_d, "w1h1b")
            w0h0 = dma_w((64, 256), w0h0_d, "w0h0")
            w0h1b = dma_w((65, 256), w0h1b_d, "w0h1b")
            wfcb = dma_w((65, 4), wfcb_d, "wfcb")

            # PE warm-up burst: ~40 dense matmuls flip the HAM clock gate
            # to K=8/8 before the phase loop starts; phases then keep PE
            # gaps under the ~3.4us re-throttle window.
            pwarm = ppool.tile([128, 256], f32, tag="pif0")
            for _w in range(40):
                nc.tensor.matmul(pwarm[:, (_w % 2) * 128:(_w % 2) * 128 + 128],
                                 wencblk[:, 0:128], wencblk[:, 0:128],
                                 start=True, stop=True)

            # persistent state
            h_all = spool.tile([128, BC], f32r)   # rows 0:64 h0, 64:128 h1
            c_all = spool.tile([128, BC], f32)    # rows 0:64 c0, 64:128 c1
            nc.vector.memset(c_all[:], 0.0)
            nc.vector.tensor_copy(h_all[:], c_all[:])

            # gate order in pytorch weights: i, f, g, o (indices 0,1,2,3)
            # encode psum layout: pif = [i | f] columns, pog = [o | g] columns
            def encode_phase(p):
                xin = xpool.tile([5, BC], f32r, tag="xin")
                nc.sync.dma_start(xin[:], xt_d.ap()[p].bitcast(f32r))
                for ch in range(CH):
                    cs = slice(ch * CW, (ch + 1) * CW)
                    pif = ppool.tile([128, 2 * CW], f32, tag=f"pif{ch}")
                    pog = ppool.tile([128, 2 * CW], f32, tag=f"pog{ch}")
                    for (ps, col, q) in ((pif, 0, 0), (pif, 1, 1),
                                         (pog, 0, 3), (pog, 1, 2)):
                        dst = ps[:, col * CW:(col + 1) * CW]
                        nc.tensor.matmul(dst, wencblk[:, q * 128:(q + 1) * 128],
                                         h_all[:, cs], start=True, stop=False)
                        nc.tensor.matmul(dst, wencx[:, q * 128:(q + 1) * 128],
                                         xin[:, cs], start=False, stop=True)
                    sif = apool.tile([128, 2 * CW], f32, tag=f"sif{ch}")
                    nc.scalar.activation(sif[:], pif[:], AF.Sigmoid)
                    sog = apool.tile([128, 2 * CW], f32, tag=f"sog{ch}")
                    nc.scalar.activation(sog[:, 0:CW], pog[:, 0:CW], AF.Sigmoid)
                    nc.scalar.activation(sog[:, CW:], pog[:, CW:], AF.Tanh)
                    t1 = tpool.tile([128, CW], f32, tag=f"t1{ch}")
                    nc.vector.tensor_tensor(t1[:], sif[:, 0:CW], sog[:, CW:], ALU.mult)
                    t2 = tpool.tile([128, CW], f32, tag=f"t2{ch}")
                    nc.vector.tensor_tensor(t2[:], sif[:, CW:], c_all[:, cs], ALU.mult)
                    rows = slice(0, 64) if p == 0 else slice(0, 128)
                    nc.vector.tensor_tensor(c_all[rows, cs], t1[rows], t2[rows], ALU.add)
                    thc = tpool.tile([128, CW], f32, tag=f"thc{ch}")
                    nc.scalar.activation(thc[:], c_all[:, cs], AF.Tanh)
                    nc.vector.tensor_tensor(h_all[:, cs], sog[:, 0:CW], thc[:], ALU.mult)

            for p in range(T_ + 1):
                encode_phase(p)

            # ---- transition to AR layout (DMA: partition ranges move) ----
            h0t = spool.tile([64, BC], f32r)
            h1b = spool.tile([65, BC], f32r)
            c0t = spool.tile([64, BC], f32)
            c1t = spool.tile([64, BC], f32)
            nc.sync.dma_start(h0t[:], h_all[0:64, :])
            nc.sync.dma_start(h1b[0:64, :], h_all[64:128, :])
            nc.sync.dma_start(h1b[64:65, :], ones_d.ap().bitcast(f32r))
            nc.sync.dma_start(c0t[:], c_all[0:64, :])
            nc.sync.dma_start(c1t[:], c_all[64:128, :])

            def ar_cell(ch, wh0, wh1b, ct, hout_t, hout_rows):
                """One LSTM cell (single layer), gates in columns, chunk ch."""
                cs = slice(ch * CW, (ch + 1) * CW)
                pq_if = ppool.tile([64, 2 * CW], f32, tag=f"pif{ch}")
                pq_og = ppool.tile([64, 2 * CW], f32, tag=f"pog{ch}")
                for (ps, col, g) in ((pq_if, 0, 0), (pq_if, 1, 1),
                                     (pq_og, 0, 2), (pq_og, 1, 3)):
                    dst = ps[:, col * CW:(col + 1) * CW]
                    nc.tensor.matmul(dst, wh0[:, g * 64:(g + 1) * 64],
                                     h0t[:, cs], start=True, stop=False)
                    nc.tensor.matmul(dst, wh1b[:, g * 64:(g + 1) * 64],
                                     h1b[:, cs], start=False, stop=True)
                sif = apool.tile([64, 2 * CW], f32, tag=f"sif{ch}")
                nc.scalar.activation(sif[:], pq_if[:], AF.Sigmoid)
                so = tpool.tile([64, CW], f32, tag=f"so{ch}")
                nc.scalar.activation(so[:], pq_og[:, 0:CW], AF.Sigmoid)
                gt = tpool.tile([64, CW], f32, tag=f"gt{ch}")
                nc.scalar.activation(gt[:], pq_og[:, CW:], AF.Tanh)
                t1 = tpool.tile([64, CW], f32, tag=f"t1{ch}")
                nc.vector.tensor_tensor(t1[:], sif[:, 0:CW], gt[:], ALU.mult)
                t2 = tpool.tile([64, CW], f32, tag=f"t2{ch}")
                nc.vector.tensor_tensor(t2[:], sif[:, CW:], ct[:, cs], ALU.mult)
                nc.vector.tensor_tensor(ct[:, cs], t1[:], t2[:], ALU.add)
                thc = tpool.tile([64, CW], f32, tag=f"thc{ch}")
                nc.scalar.activation(thc[:], ct[:, cs], AF.Tanh)
                nc.vector.tensor_tensor(hout_t[hout_rows, cs], so[:], thc[:], ALU.mult)

            for s in range(T_, T_ + STEPS_):
                for ch in range(CH):
                    cs = slice(ch * CW, (ch + 1) * CW)
                    # layer 1, step s
                    ar_cell(ch, w1h0, w1h1b, c1t, h1b, slice(0, 64))
                    # pred output (off critical path)
                    pp = ppool.tile([4, CW], f32, tag=f"pif{ch}")
                    nc.tensor.matmul(pp[:], wfcb[:], h1b[:, cs], start=True, stop=True)
                    po = tpool.tile([4, CW], f32, tag=f"po{ch}")
                    nc.vector.tensor_copy(po[:], pp[:])
                    nc.sync.dma_start(out_d.ap()[s - T_, :, cs], po[:])
                    # layer 0, step s+1 (skip on the last phase)
                    if s != T_ + STEPS_ - 1:
                        ar_cell(ch, w0h0, w0h1b, c0t, h0t, slice(0, 64))

    nc.compile()
    return nc


def _prep_inputs(x, Wih0, Whh0, bih0, bhh0, Wih1, Whh1, bih1, bhh1, Wfc, bfc,
                 T_, STEPS_):
    """Host-side layout prep. Returns (shared_weight_arrays, per_core_xt)."""
    f = np.float32
    x = np.asarray(x, f)
    Wih0, Whh0 = np.asarray(Wih0, f), np.asarray(Whh0, f)
    Wih1, Whh1 = np.asarray(Wih1, f), np.asarray(Whh1, f)
    Wfc = np.asarray(Wfc, f)
    b0 = np.asarray(bih0, f) + np.asarray(bhh0, f)   # [4H]
    b1 = np.asarray(bih1, f) + np.asarray(bhh1, f)
    bfc = np.asarray(bfc, f)

    def gate(Wm, q):
        return Wm[q * H:(q + 1) * H]

    # encode block lhsT [128k, 4 gates x 128m]
    wencblk = np.zeros((128, 512), f)
    wencx = np.zeros((5, 512), f)
    for q in range(4):
        blk = np.zeros((128, 128), f)
        blk[0:64, 0:64] = gate(Whh0, q).T
        blk[0:64, 64:128] = gate(Wih1, q).T
        blk[64:128, 64:128] = gate(Whh1, q).T
        wencblk[:, q * 128:(q + 1) * 128] = blk
        wx = np.zeros((5, 128), f)
        wx[0:4, 0:64] = gate(Wih0, q).T
        wx[4, 0:64] = gate(b0[:, None], q)[:, 0]
        wx[4, 64:128] = gate(b1[:, None], q)[:, 0]
        wencx[:, q * 128:(q + 1) * 128] = wx

    # AR weights: lhsT col blocks = gates in order (i, f, o, g), M=64 each
    Wcomb = Wih0 @ Wfc          # [4H, 64]
    b0p = b0 + Wih0 @ bfc
    order = (0, 1, 2, 3)        # pytorch q for i, f, g, o is 0,1,2,3; cols i,f,o,g
    colq = (0, 1, 3, 2)
    w1h0 = np.zeros((64, 256), f)
    w1h1b = np.zeros((65, 256), f)
    w0h0 = np.zeros((64, 256), f)
    w0h1b = np.zeros((65, 256), f)
    for j, q in enumerate(colq):
        mc = slice(j * 64, (j + 1) * 64)
        w1h0[:, mc] = gate(Wih1, q).T
        w1h1b[0:64, mc] = gate(Whh1, q).T
        w1h1b[64, mc] = gate(b1[:, None], q)[:, 0]
        w0h0[:, mc] = gate(Whh0, q).T
        w0h1b[0:64, mc] = gate(Wcomb, q).T
        w0h1b[64, mc] = gate(b0p[:, None], q)[:, 0]
    wfcb = np.zeros((65, 4), f)
    wfcb[0:64] = Wfc.T
    wfcb[64] = bfc

    shared = dict(wencblk=wencblk, wencx=wencx, w1h0=w1h0, w1h1b=w1h1b,
                  w0h0=w0h0, w0h1b=w0h1b, wfcb=wfcb,
                  ones_row=np.ones((1, BC), f))

    # per-core x-tilde: [T+1, 5, BC]; row 4 = ones; step T duplicates x_{T-1}
    xts = []
    for c in range(NCORES):
        xs = x[c * BC:(c + 1) * BC, :T_, :]          # [BC, T_, D]
        xt = np.ones((T_ + 1, 5, BC), f)
        xt[:T_, 0:4, :] = np.transpose(xs, (1, 2, 0))
        xt[T_, 0:4, :] = xs[:, T_ - 1, :].T
        xts.append(xt)
    return shared, xts


def kernel(**inputs):
    return _run(T, STEPS, **inputs)


def _run(T_, STEPS_, x, Wih0, Whh0, bih0, bhh0, Wih1, Whh1, bih1, bhh1,
         Wfc, bfc):
    from concourse.bass_utils import run_bass_kernel_spmd

    key = (T_, STEPS_)
    if key not in _cache:
        _cache[key] = _build(T_, STEPS_)
    nc = _cache[key]

    shared, xts = _prep_inputs(x, Wih0, Whh0, bih0, bhh0, Wih1, Whh1,
                               bih1, bhh1, Wfc, bfc, T_, STEPS_)
    in_maps = [{**shared, "xt": xts[c]} for c in range(NCORES)]
    res = run_bass_kernel_spmd(nc, in_maps, core_ids=list(range(NCORES)),
                               trace=TRACE)
    LAST["exec_time_ns"] = res.exec_time_ns
    LAST["res"] = res
    out = np.empty((B, STEPS_, 4), np.float32)
    for c in range(NCORES):
        # res: [STEPS, 4, BC] -> [BC, STEPS, 4]
        out[c * BC:(c + 1) * BC] = np.transpose(res.results[c]["out"], (2, 0, 1))
    return out


# revision 9
# speedup vs baseline: 1.4124x; 1.4124x over previous
"""Trainium2 Bass kernel for nn_DigitalTwinSimulator (2-layer LSTM + AR rollout).

Strategy: pure data parallel across 8 NeuronCores (batch 4096 -> 512/core).
Per core, state is kept feature-on-partitions / batch-on-free-dim.

Encode phase (t = 0..128): the two LSTM layers run in lockstep (layer 1 lags
layer 0 by one step) so all gate matmuls merge into K=128 block matmuls and
all elementwise ops are full 128-partition instructions. Biases + x
contribution enter through a K=5 matmul against [x_t; 1] (ones row built
host-side).

AR phase (steps 128..187): pred feedback is algebraically folded:
  W0x @ pred = (W0x @ Wfc) @ h1 + W0x @ bfc
so the recurrence never materializes pred; the FC head runs off the critical
path purely for output. Layers run sequentially (inherent to AR); biases ride
in a 65th row of the h1 tile (ones row).

Batch is split into 2 chunks of 256 columns that pipeline against each other
to hide the serial dependency chain latency.
"""
import os
import sys

for _p in ("/opt/trn_rl_repo", "/root/.axon_site/_ro/trn_rl_repo"):
    if os.path.isdir(_p) and _p not in sys.path:
        sys.path.append(_p)

import numpy as np

B, T, D, H, STEPS = 4096, 128, 4, 64, 60
NCORES = 8
BC = B // NCORES          # 512 batch rows per core
CH = 2                    # batch chunks per core (pipelined)
CW = BC // CH             # 256 columns per chunk

_cache = {}
TRACE = False
LAST = {}


def _build(T_, STEPS_):
    import concourse.bass as bass
    import concourse.tile as tile
    from concourse import bacc, mybir

    f32 = mybir.dt.float32
    f32r = mybir.dt.float32r
    AF = mybir.ActivationFunctionType
    ALU = mybir.AluOpType

    nc = bacc.Bacc("TRN2", target_bir_lowering=False, debug=False,
                   num_devices=NCORES)

    xt_d = nc.dram_tensor("xt", (T_ + 1, 5, BC), f32, kind="ExternalInput")
    wencblk_d = nc.dram_tensor("wencblk", (128, 512), f32, kind="ExternalInput")
    wencx_d = nc.dram_tensor("wencx", (128, 512), f32, kind="ExternalInput")
    w1h0_d = nc.dram_tensor("w1h0", (65, 256), f32, kind="ExternalInput")
    w1h1b_d = nc.dram_tensor("w1h1b", (65, 256), f32, kind="ExternalInput")
    w0h0_d = nc.dram_tensor("w0h0", (65, 256), f32, kind="ExternalInput")
    w0h1b_d = nc.dram_tensor("w0h1b", (65, 256), f32, kind="ExternalInput")
    wfcb_d = nc.dram_tensor("wfcb", (65, 4), f32, kind="ExternalInput")
    ones_d = nc.dram_tensor("ones_row", (1, BC), f32, kind="ExternalInput")
    out_d = nc.dram_tensor("out", (STEPS_, 4, BC), f32, kind="ExternalOutput")

    with tile.TileContext(nc) as tc:
        with tc.tile_pool(name="const", bufs=1) as cpool, \
             tc.tile_pool(name="state", bufs=1) as spool, \
             tc.tile_pool(name="xin", bufs=4) as xpool, \
             tc.tile_pool(name="act", bufs=3) as apool, \
             tc.tile_pool(name="tmp", bufs=3) as tpool, \
             tc.tile_pool(name="psum", bufs=2, space="PSUM") as ppool:

            def dma_w(shape, src, tag, dt=f32r):
                t = cpool.tile(list(shape), dt, tag=tag)
                nc.sync.dma_start(t[:], src.ap().bitcast(dt) if dt is f32r else src.ap())
                return t

            wencblk = dma_w((128, 512), wencblk_d, "wencblk")
            wencx = dma_w((128, 512), wencx_d, "wencx")
            w1h0 = dma_w((65, 256), w1h0_d, "w1h0")
            w1h1b = dma_w((65, 256), w1h1b_d, "w1h1b")
            w0h0 = dma_w((65, 256), w0h0_d, "w0h0")
            w0h1b = dma_w((65, 256), w0h1b_d, "w0h1b")
            wfcb = dma_w((65, 4), wfcb_d, "wfcb")

            # PE warm-up burst: ~40 dense matmuls flip the HAM clock gate
            # to K=8/8 before the phase loop starts; phases then keep PE
            # gaps under the ~3.4us re-throttle window.
            pwarm = ppool.tile([128, 256], f32, tag="pif0")
            for _w in range(40):
                nc.tensor.matmul(pwarm[:, (_w % 2) * 128:(_w % 2) * 128 + 128],
                                 wencblk[:, 0:128], wencblk[:, 0:128],
                                 start=True, stop=True)

            # persistent state
            h_all = spool.tile([128, BC], f32r)   # rows 0:64 h0, 64:128 h1
            c_all = spool.tile([128, BC], f32)    # rows 0:64 c0, 64:128 c1
            nc.vector.memset(c_all[:], 0.0)
            nc.vector.tensor_copy(h_all[:], c_all[:])
            # x-tilde staging tiles: rows 5:128 stay zero so the x matmul can
            # run at K=128 (uniform K avoids the PE weight-size-switch stall)
            xins = []
            for _i in range(3):
                xt_t = spool.tile([128, BC], f32r, tag=f"xin{_i}")
                nc.vector.tensor_copy(xt_t[:], c_all[:])
                xins.append(xt_t)

            # gate order in pytorch weights: i, f, g, o (indices 0,1,2,3)
            # encode psum layout: pif = [i | f] columns, pog = [o | g] columns
            def encode_phase(p):
                xin = xins[p % 3]
                nc.sync.dma_start(xin[0:5, :], xt_d.ap()[p].bitcast(f32r))
                for ch in range(CH):
                    cs = slice(ch * CW, (ch + 1) * CW)
                    pif = ppool.tile([128, 2 * CW], f32, tag=f"pif{ch}")
                    pog = ppool.tile([128, 2 * CW], f32, tag=f"pog{ch}")
                    for (ps, col, q) in ((pif, 0, 0), (pif, 1, 1),
                                         (pog, 0, 3), (pog, 1, 2)):
                        dst = ps[:, col * CW:(col + 1) * CW]
                        nc.tensor.matmul(dst, wencblk[:, q * 128:(q + 1) * 128],
                                         h_all[:, cs], start=True, stop=False)
                        nc.tensor.matmul(dst, wencx[:, q * 128:(q + 1) * 128],
                                         xin[:, cs], start=False, stop=True)
                    sif = apool.tile([128, 2 * CW], f32, tag=f"sif{ch}")
                    nc.scalar.activation(sif[:], pif[:], AF.Sigmoid)
                    sog = apool.tile([128, 2 * CW], f32, tag=f"sog{ch}")
                    nc.scalar.activation(sog[:, 0:CW], pog[:, 0:CW], AF.Sigmoid)
                    nc.scalar.activation(sog[:, CW:], pog[:, CW:], AF.Tanh)
                    t1 = tpool.tile([128, CW], f32, tag=f"t1{ch}")
                    nc.vector.tensor_tensor(t1[:], sif[:, 0:CW], sog[:, CW:], ALU.mult)
                    t2 = tpool.tile([128, CW], f32, tag=f"t2{ch}")
                    nc.vector.tensor_tensor(t2[:], sif[:, CW:], c_all[:, cs], ALU.mult)
                    rows = slice(0, 64) if p == 0 else slice(0, 128)
                    nc.vector.tensor_tensor(c_all[rows, cs], t1[rows], t2[rows], ALU.add)
                    thc = tpool.tile([128, CW], f32, tag=f"thc{ch}")
                    nc.scalar.activation(thc[:], c_all[:, cs], AF.Tanh)
                    nc.vector.tensor_tensor(h_all[:, cs], sog[:, 0:CW], thc[:], ALU.mult)

            for p in range(T_ + 1):
                encode_phase(p)

            # ---- transition to AR layout (DMA: partition ranges move) ----
            h0t = spool.tile([65, BC], f32r)
            h1b = spool.tile([65, BC], f32r)
            c0t = spool.tile([64, BC], f32)
            c1t = spool.tile([64, BC], f32)
            nc.sync.dma_start(h0t[0:64, :], h_all[0:64, :])
            nc.sync.dma_start(h0t[64:65, :], ones_d.ap().bitcast(f32r))
            nc.sync.dma_start(h1b[0:64, :], h_all[64:128, :])
            nc.sync.dma_start(h1b[64:65, :], ones_d.ap().bitcast(f32r))
            nc.sync.dma_start(c0t[:], c_all[0:64, :])
            nc.sync.dma_start(c1t[:], c_all[64:128, :])

            def ar_cell(ch, wh0, wh1b, ct, hout_t, hout_rows):
                """One LSTM cell (single layer), gates in columns, chunk ch."""
                cs = slice(ch * CW, (ch + 1) * CW)
                pq_if = ppool.tile([64, 2 * CW], f32, tag=f"pif{ch}")
                pq_og = ppool.tile([64, 2 * CW], f32, tag=f"pog{ch}")
                for (ps, col, g) in ((pq_if, 0, 0), (pq_if, 1, 1),
                                     (pq_og, 0, 2), (pq_og, 1, 3)):
                    dst = ps[:, col * CW:(col + 1) * CW]
                    nc.tensor.matmul(dst, wh0[:, g * 64:(g + 1) * 64],
                                     h0t[:, cs], start=True, stop=False)
                    nc.tensor.matmul(dst, wh1b[:, g * 64:(g + 1) * 64],
                                     h1b[:, cs], start=False, stop=True)
                sif = apool.tile([64, 2 * CW], f32, tag=f"sif{ch}")
                nc.scalar.activation(sif[:], pq_if[:], AF.Sigmoid)
                so = tpool.tile([64, CW], f32, tag=f"so{ch}")
                nc.scalar.activation(so[:], pq_og[:, 0:CW], AF.Sigmoid)
                gt = tpool.tile([64, CW], f32, tag=f"gt{ch}")
                nc.scalar.activation(gt[:], pq_og[:, CW:], AF.Tanh)
                t1 = tpool.tile([64, CW], f32, tag=f"t1{ch}")
                nc.vector.tensor_tensor(t1[:], sif[:, 0:CW], gt[:], ALU.mult)
                t2 = tpool.tile([64, CW], f32, tag=f"t2{ch}")
                nc.vector.tensor_tensor(t2[:], sif[:, CW:], ct[:, cs], ALU.mult)
                nc.vector.tensor_tensor(ct[:, cs], t1[:], t2[:], ALU.add)
                thc = tpool.tile([64, CW], f32, tag=f"thc{ch}")
                nc.scalar.activation(thc[:], ct[:, cs], AF.Tanh)
                nc.vector.tensor_tensor(hout_t[hout_rows, cs], so[:], thc[:], ALU.mult)

            for s in range(T_, T_ + STEPS_):
                for ch in range(CH):
                    cs = slice(ch * CW, (ch + 1) * CW)
                    # layer 1, step s
                    ar_cell(ch, w1h0, w1h1b, c1t, h1b, slice(0, 64))
                    # pred output (off critical path)
                    pp = ppool.tile([4, CW], f32, tag=f"pif{ch}")
                    nc.tensor.matmul(pp[:], wfcb[:], h1b[:, cs], start=True, stop=True)
                    po = tpool.tile([4, CW], f32, tag=f"po{ch}")
                    nc.vector.tensor_copy(po[:], pp[:])
                    nc.sync.dma_start(out_d.ap()[s - T_, :, cs], po[:])
                    # layer 0, step s+1 (skip on the last phase)
                    if s != T_ + STEPS_ - 1:
                        ar_cell(ch, w0h0, w0h1b, c0t, h0t, slice(0, 64))

    nc.compile()
    return nc


def _prep_inputs(x, Wih0, Whh0, bih0, bhh0, Wih1, Whh1, bih1, bhh1, Wfc, bfc,
                 T_, STEPS_):
    """Host-side layout prep. Returns (shared_weight_arrays, per_core_xt)."""
    f = np.float32
    x = np.asarray(x, f)
    Wih0, Whh0 = np.asarray(Wih0, f), np.asarray(Whh0, f)
    Wih1, Whh1 = np.asarray(Wih1, f), np.asarray(Whh1, f)
    Wfc = np.asarray(Wfc, f)
    b0 = np.asarray(bih0, f) + np.asarray(bhh0, f)   # [4H]
    b1 = np.asarray(bih1, f) + np.asarray(bhh1, f)
    bfc = np.asarray(bfc, f)

    def gate(Wm, q):
        return Wm[q * H:(q + 1) * H]

    # encode block lhsT [128k, 4 gates x 128m]
    wencblk = np.zeros((128, 512), f)
    wencx = np.zeros((128, 512), f)
    for q in range(4):
        blk = np.zeros((128, 128), f)
        blk[0:64, 0:64] = gate(Whh0, q).T
        blk[0:64, 64:128] = gate(Wih1, q).T
        blk[64:128, 64:128] = gate(Whh1, q).T
        wencblk[:, q * 128:(q + 1) * 128] = blk
        wx = np.zeros((5, 128), f)
        wx[0:4, 0:64] = gate(Wih0, q).T
        wx[4, 0:64] = gate(b0[:, None], q)[:, 0]
        wx[4, 64:128] = gate(b1[:, None], q)[:, 0]
        wencx[0:5, q * 128:(q + 1) * 128] = wx

    # AR weights: lhsT col blocks = gates in order (i, f, o, g), M=64 each
    Wcomb = Wih0 @ Wfc          # [4H, 64]
    b0p = b0 + Wih0 @ bfc
    order = (0, 1, 2, 3)        # pytorch q for i, f, g, o is 0,1,2,3; cols i,f,o,g
    colq = (0, 1, 3, 2)
    w1h0 = np.zeros((65, 256), f)
    w1h1b = np.zeros((65, 256), f)
    w0h0 = np.zeros((65, 256), f)
    w0h1b = np.zeros((65, 256), f)
    for j, q in enumerate(colq):
        mc = slice(j * 64, (j + 1) * 64)
        w1h0[0:64, mc] = gate(Wih1, q).T
        w1h1b[0:64, mc] = gate(Whh1, q).T
        w1h1b[64, mc] = gate(b1[:, None], q)[:, 0]
        w0h0[0:64, mc] = gate(Whh0, q).T
        w0h1b[0:64, mc] = gate(Wcomb, q).T
        w0h1b[64, mc] = gate(b0p[:, None], q)[:, 0]
    wfcb = np.zeros((65, 4), f)
    wfcb[0:64] = Wfc.T
    wfcb[64] = bfc

    shared = dict(wencblk=wencblk, wencx=wencx, w1h0=w1h0, w1h1b=w1h1b,
                  w0h0=w0h0, w0h1b=w0h1b, wfcb=wfcb,
                  ones_row=np.ones((1, BC), f))

    # per-core x-tilde: [T+1, 5, BC]; row 4 = ones; step T duplicates x_{T-1}
    xts = []
    for c in range(NCORES):
        xs = x[c * BC:(c + 1) * BC, :T_, :]          # [BC, T_, D]
        xt = np.ones((T_ + 1, 5, BC), f)
        xt[:T_, 0:4, :] = np.transpose(xs, (1, 2, 0))
        xt[T_, 0:4, :] = xs[:, T_ - 1, :].T
        xts.append(xt)
    return shared, xts


def kernel(**inputs):
    return _run(T, STEPS, **inputs)


def _run(T_, STEPS_, x, Wih0, Whh0, bih0, bhh0, Wih1, Whh1, bih1, bhh1,
         Wfc, bfc):
    from concourse.bass_utils import run_bass_kernel_spmd

    key = (T_, STEPS_)
    if key not in _cache:
        _cache[key] = _build(T_, STEPS_)
    nc = _cache[key]

    shared, xts = _prep_inputs(x, Wih0, Whh0, bih0, bhh0, Wih1, Whh1,
                               bih1, bhh1, Wfc, bfc, T_, STEPS_)
    in_maps = [{**shared, "xt": xts[c]} for c in range(NCORES)]
    res = run_bass_kernel_spmd(nc, in_maps, core_ids=list(range(NCORES)),
                               trace=TRACE)
    LAST["exec_time_ns"] = res.exec_time_ns
    LAST["res"] = res
    out = np.empty((B, STEPS_, 4), np.float32)
    for c in range(NCORES):
        # res: [STEPS, 4, BC] -> [BC, STEPS, 4]
        out[c * BC:(c + 1) * BC] = np.transpose(res.results[c]["out"], (2, 0, 1))
    return out


# revision 10
# speedup vs baseline: 1.4134x; 1.0006x over previous
"""Trainium2 Bass kernel for nn_DigitalTwinSimulator (2-layer LSTM + AR rollout).

Strategy: pure data parallel across 8 NeuronCores (batch 4096 -> 512/core).
Per core, state is kept feature-on-partitions / batch-on-free-dim.

Encode phase (t = 0..128): the two LSTM layers run in lockstep (layer 1 lags
layer 0 by one step) so all gate matmuls merge into K=128 block matmuls and
all elementwise ops are full 128-partition instructions. Biases + x
contribution enter through a K=5 matmul against [x_t; 1] (ones row built
host-side).

AR phase (steps 128..187): pred feedback is algebraically folded:
  W0x @ pred = (W0x @ Wfc) @ h1 + W0x @ bfc
so the recurrence never materializes pred; the FC head runs off the critical
path purely for output. Layers run sequentially (inherent to AR); biases ride
in a 65th row of the h1 tile (ones row).

Batch is split into 2 chunks of 256 columns that pipeline against each other
to hide the serial dependency chain latency.
"""
import os
import sys

for _p in ("/opt/trn_rl_repo", "/root/.axon_site/_ro/trn_rl_repo"):
    if os.path.isdir(_p) and _p not in sys.path:
        sys.path.append(_p)

import numpy as np

B, T, D, H, STEPS = 4096, 128, 4, 64, 60
NCORES = 8
BC = B // NCORES          # 512 batch rows per core
CH = 2                    # batch chunks per core (pipelined)
CW = BC // CH             # 256 columns per chunk

_cache = {}
TRACE = False
LAST = {}


def _build(T_, STEPS_):
    import concourse.bass as bass
    import concourse.tile as tile
    from concourse import bacc, mybir

    f32 = mybir.dt.float32
    f32r = mybir.dt.float32r
    AF = mybir.ActivationFunctionType
    ALU = mybir.AluOpType

    nc = bacc.Bacc("TRN2", target_bir_lowering=False, debug=False,
                   num_devices=NCORES)

    xt_d = nc.dram_tensor("xt", (T_ + 1, 5, BC), f32, kind="ExternalInput")
    wencblk_d = nc.dram_tensor("wencblk", (128, 512), f32, kind="ExternalInput")
    wencx_d = nc.dram_tensor("wencx", (128, 512), f32, kind="ExternalInput")
    w1h0_d = nc.dram_tensor("w1h0", (65, 256), f32, kind="ExternalInput")
    w1h1b_d = nc.dram_tensor("w1h1b", (65, 256), f32, kind="ExternalInput")
    w0h0_d = nc.dram_tensor("w0h0", (65, 256), f32, kind="ExternalInput")
    w0h1b_d = nc.dram_tensor("w0h1b", (65, 256), f32, kind="ExternalInput")
    wfcb_d = nc.dram_tensor("wfcb", (65, 4), f32, kind="ExternalInput")
    ones_d = nc.dram_tensor("ones_row", (1, BC), f32, kind="ExternalInput")
    out_d = nc.dram_tensor("out", (STEPS_, 4, BC), f32, kind="ExternalOutput")

    with tile.TileContext(nc) as tc:
        with tc.tile_pool(name="const", bufs=1) as cpool, \
             tc.tile_pool(name="state", bufs=1) as spool, \
             tc.tile_pool(name="act", bufs=3) as apool, \
             tc.tile_pool(name="tmp", bufs=3) as tpool, \
             tc.tile_pool(name="psum", bufs=2, space="PSUM") as ppool:

            def dma_w(shape, src, tag, dt=f32r):
                t = cpool.tile(list(shape), dt, tag=tag)
                nc.sync.dma_start(t[:], src.ap().bitcast(dt))
                return t

            wencblk = dma_w((128, 512), wencblk_d, "wencblk")
            wencx = dma_w((128, 512), wencx_d, "wencx")
            w1h0 = dma_w((65, 256), w1h0_d, "w1h0")
            w1h1b = dma_w((65, 256), w1h1b_d, "w1h1b")
            w0h0 = dma_w((65, 256), w0h0_d, "w0h0")
            w0h1b = dma_w((65, 256), w0h1b_d, "w0h1b")
            wfcb = dma_w((65, 4), wfcb_d, "wfcb")

            # PE warm-up burst to flip the HAM clock gate before phase 0
            pwarm = ppool.tile([128, 256], f32, tag="pif0")
            for _w in range(40):
                nc.tensor.matmul(pwarm[:, (_w % 2) * 128:(_w % 2) * 128 + 128],
                                 wencblk[:, 0:128], wencblk[:, 0:128],
                                 start=True, stop=True)

            # per-chunk persistent state (separate tiles so the two batch
            # chunks share no tiles -> no false cross-chunk dependencies)
            zsrc = spool.tile([128, BC], f32, tag="zsrc")
            nc.vector.memset(zsrc[:], 0.0)
            h_st, c_st, xins = [], [], []
            for ch in range(CH):
                h = spool.tile([128, CW], f32r, tag=f"h_st{ch}")
                nc.vector.tensor_copy(h[:], zsrc[:, 0:CW])
                h_st.append(h)
                c = spool.tile([128, CW], f32, tag=f"c_st{ch}")
                nc.vector.memset(c[:], 0.0)
                c_st.append(c)
            # x staging: rows 5:128 stay zero => x matmul runs at K=128
            for _i in range(3):
                xt_t = spool.tile([128, BC], f32r, tag=f"xin{_i}")
                nc.vector.tensor_copy(xt_t[:], zsrc[:])
                xins.append(xt_t)

            def encode_phase(p):
                xin = xins[p % 3]
                nc.sync.dma_start(xin[0:5, :], xt_d.ap()[p].bitcast(f32r))
                for ch in range(CH):
                    cs = slice(ch * CW, (ch + 1) * CW)
                    h, c = h_st[ch], c_st[ch]
                    pif = ppool.tile([128, 2 * CW], f32, tag=f"pif{ch}")
                    pog = ppool.tile([128, 2 * CW], f32, tag=f"pog{ch}")
                    for (ps, col, q) in ((pif, 0, 0), (pif, 1, 1),
                                         (pog, 0, 3), (pog, 1, 2)):
                        dst = ps[:, col * CW:(col + 1) * CW]
                        nc.tensor.matmul(dst, wencblk[:, q * 128:(q + 1) * 128],
                                         h[:], start=True, stop=False)
                        nc.tensor.matmul(dst, wencx[:, q * 128:(q + 1) * 128],
                                         xin[:, cs], start=False, stop=True)
                    sif = apool.tile([128, 2 * CW], f32, tag=f"sif{ch}")
                    nc.scalar.activation(sif[:], pif[:], AF.Sigmoid)
                    sog = apool.tile([128, 2 * CW], f32, tag=f"sog{ch}")
                    nc.scalar.activation(sog[:, 0:CW], pog[:, 0:CW], AF.Sigmoid)
                    nc.scalar.activation(sog[:, CW:], pog[:, CW:], AF.Tanh)
                    t1 = tpool.tile([128, CW], f32, tag=f"t1{ch}")
                    nc.vector.tensor_tensor(t1[:], sif[:, 0:CW], sog[:, CW:], ALU.mult)
                    t2 = tpool.tile([128, CW], f32, tag=f"t2{ch}")
                    nc.vector.tensor_tensor(t2[:], sif[:, CW:], c[:], ALU.mult)
                    rows = slice(0, 64) if p == 0 else slice(0, 128)
                    nc.vector.tensor_tensor(c[rows, :], t1[rows, :], t2[rows, :], ALU.add)
                    thc = tpool.tile([128, CW], f32, tag=f"thc{ch}")
                    nc.scalar.activation(thc[:], c[:], AF.Tanh)
                    nc.vector.tensor_tensor(h[:], sog[:, 0:CW], thc[:], ALU.mult)

            for p in range(T_ + 1):
                encode_phase(p)

            # ---- transition to AR layout (per-chunk tiles) ----
            h0t, h1b, c0t, c1t = [], [], [], []
            for ch in range(CH):
                cs = slice(ch * CW, (ch + 1) * CW)
                a = spool.tile([65, CW], f32r, tag=f"h0t{ch}")
                nc.sync.dma_start(a[0:64, :], h_st[ch][0:64, :])
                nc.sync.dma_start(a[64:65, :], ones_d.ap()[0:1, cs].bitcast(f32r))
                h0t.append(a)
                b = spool.tile([65, CW], f32r, tag=f"h1b{ch}")
                nc.sync.dma_start(b[0:64, :], h_st[ch][64:128, :])
                nc.sync.dma_start(b[64:65, :], ones_d.ap()[0:1, cs].bitcast(f32r))
                h1b.append(b)
                c0 = spool.tile([64, CW], f32, tag=f"c0t{ch}")
                nc.sync.dma_start(c0[:], c_st[ch][0:64, :])
                c0t.append(c0)
                c1 = spool.tile([64, CW], f32, tag=f"c1t{ch}")
                nc.sync.dma_start(c1[:], c_st[ch][64:128, :])
                c1t.append(c1)

            def ar_cell(ch, wh0, wh1b, ct, hout_t, hout_rows):
                """One LSTM cell (single layer), gates in columns, chunk ch."""
                pq_if = ppool.tile([64, 2 * CW], f32, tag=f"pif{ch}")
                pq_og = ppool.tile([64, 2 * CW], f32, tag=f"pog{ch}")
                for (ps, col, g) in ((pq_if, 0, 0), (pq_if, 1, 1),
                                     (pq_og, 0, 2), (pq_og, 1, 3)):
                    dst = ps[:, col * CW:(col + 1) * CW]
                    nc.tensor.matmul(dst, wh0[:, g * 64:(g + 1) * 64],
                                     h0t[ch][:], start=True, stop=False)
                    nc.tensor.matmul(dst, wh1b[:, g * 64:(g + 1) * 64],
                                     h1b[ch][:], start=False, stop=True)
                sif = apool.tile([64, 2 * CW], f32, tag=f"sif{ch}")
                nc.scalar.activation(sif[:], pq_if[:], AF.Sigmoid)
                so = tpool.tile([64, CW], f32, tag=f"so{ch}")
                nc.scalar.activation(so[:], pq_og[:, 0:CW], AF.Sigmoid)
                gt = tpool.tile([64, CW], f32, tag=f"gt{ch}")
                nc.scalar.activation(gt[:], pq_og[:, CW:], AF.Tanh)
                t1 = tpool.tile([64, CW], f32, tag=f"t1{ch}")
                nc.vector.tensor_tensor(t1[:], sif[:, 0:CW], gt[:], ALU.mult)
                t2 = tpool.tile([64, CW], f32, tag=f"t2{ch}")
                nc.vector.tensor_tensor(t2[:], sif[:, CW:], ct[:], ALU.mult)
                nc.vector.tensor_tensor(ct[:], t1[:], t2[:], ALU.add)
                thc = tpool.tile([64, CW], f32, tag=f"thc{ch}")
                nc.scalar.activation(thc[:], ct[:], AF.Tanh)
                nc.vector.tensor_tensor(hout_t[hout_rows, :], so[:], thc[:], ALU.mult)

            for s in range(T_, T_ + STEPS_):
                for ch in range(CH):
                    cs = slice(ch * CW, (ch + 1) * CW)
                    # layer 1, step s
                    ar_cell(ch, w1h0, w1h1b, c1t[ch], h1b[ch], slice(0, 64))
                    # pred output (off critical path)
                    pp = ppool.tile([4, CW], f32, tag=f"pif{ch}")
                    nc.tensor.matmul(pp[:], wfcb[:], h1b[ch][:], start=True, stop=True)
                    po = tpool.tile([4, CW], f32, tag=f"po{ch}")
                    nc.vector.tensor_copy(po[:], pp[:])
                    nc.sync.dma_start(out_d.ap()[s - T_, :, cs], po[:])
                    # layer 0, step s+1 (skip on the last phase)
                    if s != T_ + STEPS_ - 1:
                        ar_cell(ch, w0h0, w0h1b, c0t[ch], h0t[ch], slice(0, 64))

    nc.compile()
    return nc


def _prep_inputs(x, Wih0, Whh0, bih0, bhh0, Wih1, Whh1, bih1, bhh1, Wfc, bfc,
                 T_, STEPS_):
    """Host-side layout prep. Returns (shared_weight_arrays, per_core_xt)."""
    f = np.float32
    x = np.asarray(x, f)
    Wih0, Whh0 = np.asarray(Wih0, f), np.asarray(Whh0, f)
    Wih1, Whh1 = np.asarray(Wih1, f), np.asarray(Whh1, f)
    Wfc = np.asarray(Wfc, f)
    b0 = np.asarray(bih0, f) + np.asarray(bhh0, f)   # [4H]
    b1 = np.asarray(bih1, f) + np.asarray(bhh1, f)
    bfc = np.asarray(bfc, f)

    def gate(Wm, q):
        return Wm[q * H:(q + 1) * H]

    # encode block lhsT [128k, 4 gates x 128m]
    wencblk = np.zeros((128, 512), f)
    wencx = np.zeros((128, 512), f)
    for q in range(4):
        blk = np.zeros((128, 128), f)
        blk[0:64, 0:64] = gate(Whh0, q).T
        blk[0:64, 64:128] = gate(Wih1, q).T
        blk[64:128, 64:128] = gate(Whh1, q).T
        wencblk[:, q * 128:(q + 1) * 128] = blk
        wx = np.zeros((5, 128), f)
        wx[0:4, 0:64] = gate(Wih0, q).T
        wx[4, 0:64] = gate(b0[:, None], q)[:, 0]
        wx[4, 64:128] = gate(b1[:, None], q)[:, 0]
        wencx[0:5, q * 128:(q + 1) * 128] = wx

    # AR weights: lhsT col blocks = gates in order (i, f, o, g), M=64 each
    Wcomb = Wih0 @ Wfc          # [4H, 64]
    b0p = b0 + Wih0 @ bfc
    order = (0, 1, 2, 3)        # pytorch q for i, f, g, o is 0,1,2,3; cols i,f,o,g
    colq = (0, 1, 3, 2)
    w1h0 = np.zeros((65, 256), f)
    w1h1b = np.zeros((65, 256), f)
    w0h0 = np.zeros((65, 256), f)
    w0h1b = np.zeros((65, 256), f)
    for j, q in enumerate(colq):
        mc = slice(j * 64, (j + 1) * 64)
        w1h0[0:64, mc] = gate(Wih1, q).T
        w1h1b[0:64, mc] = gate(Whh1, q).T
        w1h1b[64, mc] = gate(b1[:, None], q)[:, 0]
        w0h0[0:64, mc] = gate(Whh0, q).T
        w0h1b[0:64, mc] = gate(Wcomb, q).T
        w0h1b[64, mc] = gate(b0p[:, None], q)[:, 0]
    wfcb = np.zeros((65, 4), f)
    wfcb[0:64] = Wfc.T
    wfcb[64] = bfc

    shared = dict(wencblk=wencblk, wencx=wencx, w1h0=w1h0, w1h1b=w1h1b,
                  w0h0=w0h0, w0h1b=w0h1b, wfcb=wfcb,
                  ones_row=np.ones((1, BC), f))

    # per-core x-tilde: [T+1, 5, BC]; row 4 = ones; step T duplicates x_{T-1}
    xts = []
    for c in range(NCORES):
        xs = x[c * BC:(c + 1) * BC, :T_, :]          # [BC, T_, D]
        xt = np.ones((T_ + 1, 5, BC), f)
        xt[:T_, 0:4, :] = np.transpose(xs, (1, 2, 0))
        xt[T_, 0:4, :] = xs[:, T_ - 1, :].T
        xts.append(xt)
    return shared, xts


def kernel(**inputs):
    return _run(T, STEPS, **inputs)


def _run(T_, STEPS_, x, Wih0, Whh0, bih0, bhh0, Wih1, Whh1, bih1, bhh1,
         Wfc, bfc):
    from concourse.bass_utils import run_bass_kernel_spmd

    key = (T_, STEPS_)
    if key not in _cache:
        _cache[key] = _build(T_, STEPS_)
    nc = _cache[key]

    shared, xts = _prep_inputs(x, Wih0, Whh0, bih0, bhh0, Wih1, Whh1,
                               bih1, bhh1, Wfc, bfc, T_, STEPS_)
    in_maps = [{**shared, "xt": xts[c]} for c in range(NCORES)]
    res = run_bass_kernel_spmd(nc, in_maps, core_ids=list(range(NCORES)),
                               trace=TRACE)
    LAST["exec_time_ns"] = res.exec_time_ns
    LAST["res"] = res
    out = np.empty((B, STEPS_, 4), np.float32)
    for c in range(NCORES):
        # res: [STEPS, 4, BC] -> [BC, STEPS, 4]
        out[c * BC:(c + 1) * BC] = np.transpose(res.results[c]["out"], (2, 0, 1))
    return out


# revision 14
# speedup vs baseline: 1.5885x; 1.1239x over previous
"""Trainium2 Bass kernel for nn_DigitalTwinSimulator (2-layer LSTM + AR rollout).

Strategy: pure data parallel across 8 NeuronCores (batch 4096 -> 512/core).
Per core, state is kept feature-on-partitions / batch-on-free-dim.

Encode phase (t = 0..128): the two LSTM layers run in lockstep (layer 1 lags
layer 0 by one step) so all gate matmuls merge into K=128 block matmuls and
all elementwise ops are full 128-partition instructions. Biases + x
contribution enter through a K=5 matmul against [x_t; 1] (ones row built
host-side).

AR phase (steps 128..187): pred feedback is algebraically folded:
  W0x @ pred = (W0x @ Wfc) @ h1 + W0x @ bfc
so the recurrence never materializes pred; the FC head runs off the critical
path purely for output. Layers run sequentially (inherent to AR); biases ride
in a 65th row of the h1 tile (ones row).

Batch is split into 2 chunks of 256 columns that pipeline against each other
to hide the serial dependency chain latency.
"""
import os
import sys

for _p in ("/opt/trn_rl_repo", "/root/.axon_site/_ro/trn_rl_repo"):
    if os.path.isdir(_p) and _p not in sys.path:
        sys.path.append(_p)

import numpy as np

B, T, D, H, STEPS = 4096, 128, 4, 64, 60
NCORES = 8
BC = B // NCORES          # 512 batch rows per core
CH = 2                    # batch chunks per core (pipelined)
CW = BC // CH             # 256 columns per chunk

_cache = {}
TRACE = False
LAST = {}


def _build(T_, STEPS_):
    import concourse.bass as bass
    import concourse.tile as tile
    from concourse import bacc, mybir

    f32 = mybir.dt.float32
    f32r = mybir.dt.float32r
    AF = mybir.ActivationFunctionType
    ALU = mybir.AluOpType

    nc = bacc.Bacc("TRN2", target_bir_lowering=False, debug=False,
                   num_devices=NCORES)

    xt_d = nc.dram_tensor("xt", (T_ + 1, 5, BC), f32, kind="ExternalInput")
    wencblk_d = nc.dram_tensor("wencblk", (128, 512), f32, kind="ExternalInput")
    wencx_d = nc.dram_tensor("wencx", (128, 512), f32, kind="ExternalInput")
    w1h0_d = nc.dram_tensor("w1h0", (65, 256), f32, kind="ExternalInput")
    w1h1b_d = nc.dram_tensor("w1h1b", (65, 256), f32, kind="ExternalInput")
    w0h0_d = nc.dram_tensor("w0h0", (65, 256), f32, kind="ExternalInput")
    w0h1b_d = nc.dram_tensor("w0h1b", (65, 256), f32, kind="ExternalInput")
    wfcb_d = nc.dram_tensor("wfcb", (65, 4), f32, kind="ExternalInput")
    ones_d = nc.dram_tensor("ones_row", (1, BC), f32, kind="ExternalInput")
    out_d = nc.dram_tensor("out", (STEPS_, 4, BC), f32, kind="ExternalOutput")

    with tile.TileContext(nc) as tc:
        with tc.tile_pool(name="const", bufs=1) as cpool, \
             tc.tile_pool(name="state", bufs=1) as spool, \
             tc.tile_pool(name="act", bufs=3) as apool, \
             tc.tile_pool(name="tmp", bufs=3) as tpool, \
             tc.tile_pool(name="psum", bufs=2, space="PSUM") as ppool:

            def dma_w(shape, src, tag, dt=f32r):
                t = cpool.tile(list(shape), dt, tag=tag)
                nc.sync.dma_start(t[:], src.ap().bitcast(dt))
                return t

            wencblk = dma_w((128, 512), wencblk_d, "wencblk")
            wencx = dma_w((128, 512), wencx_d, "wencx")
            w1h0 = dma_w((65, 256), w1h0_d, "w1h0")
            w1h1b = dma_w((65, 256), w1h1b_d, "w1h1b")
            w0h0 = dma_w((65, 256), w0h0_d, "w0h0")
            w0h1b = dma_w((65, 256), w0h1b_d, "w0h1b")
            wfcb = dma_w((65, 4), wfcb_d, "wfcb")

            # PE warm-up burst to flip the HAM clock gate before phase 0
            pwarm = ppool.tile([128, 256], f32, tag="pif0")
            for _w in range(40):
                nc.tensor.matmul(pwarm[:, (_w % 2) * 128:(_w % 2) * 128 + 128],
                                 wencblk[:, 0:128], wencblk[:, 0:128],
                                 start=True, stop=True)

            # per-chunk persistent state (separate tiles so the two batch
            # chunks share no tiles -> no false cross-chunk dependencies)
            zsrc = spool.tile([128, BC], f32, tag="zsrc")
            nc.vector.memset(zsrc[:], 0.0)
            h_st, c_st, xins = [], [], []
            for ch in range(CH):
                h = spool.tile([128, CW], f32r, tag=f"h_st{ch}")
                nc.vector.tensor_copy(h[:], zsrc[:, 0:CW])
                h_st.append(h)
                c = spool.tile([128, CW], f32, tag=f"c_st{ch}")
                nc.vector.memset(c[:], 0.0)
                c_st.append(c)
            # x staging: rows 5:128 stay zero => x matmul runs at K=128
            for _i in range(3):
                xt_t = spool.tile([128, BC], f32r, tag=f"xin{_i}")
                nc.vector.tensor_copy(xt_t[:], zsrc[:])
                xins.append(xt_t)

            def encode_phase(p):
                xin = xins[p % 3]
                nc.sync.dma_start(xin[0:5, :], xt_d.ap()[p].bitcast(f32r))
                for ch in range(CH):
                    cs = slice(ch * CW, (ch + 1) * CW)
                    h, c = h_st[ch], c_st[ch]
                    pif = ppool.tile([128, 2 * CW], f32, tag=f"pif{ch}")
                    pog = ppool.tile([128, 2 * CW], f32, tag=f"pog{ch}")
                    for (ps, col, q) in ((pif, 0, 0), (pif, 1, 1),
                                         (pog, 1, 2), (pog, 0, 3)):
                        dst = ps[:, col * CW:(col + 1) * CW]
                        nc.tensor.matmul(dst, wencblk[:, q * 128:(q + 1) * 128],
                                         h[:], start=True, stop=False)
                        nc.tensor.matmul(dst, wencx[:, q * 128:(q + 1) * 128],
                                         xin[:, cs], start=False, stop=True)
                    sif = apool.tile([128, 2 * CW], f32, tag=f"sif{ch}")
                    nc.scalar.activation(sif[:], pif[:], AF.Sigmoid)
                    sog = apool.tile([128, 2 * CW], f32, tag=f"sog{ch}")
                    nc.scalar.activation(sog[:, CW:], pog[:, CW:], AF.Tanh)
                    t2 = tpool.tile([128, CW], f32, tag=f"t2{ch}")
                    nc.vector.tensor_tensor(t2[:], sif[:, CW:], c[:], ALU.mult)
                    nc.scalar.activation(sog[:, 0:CW], pog[:, 0:CW], AF.Sigmoid)
                    t1 = tpool.tile([128, CW], f32, tag=f"t1{ch}")
                    nc.vector.tensor_tensor(t1[:], sif[:, 0:CW], sog[:, CW:], ALU.mult)
                    rows = slice(0, 64) if p == 0 else slice(0, 128)
                    nc.vector.tensor_tensor(c[rows, :], t1[rows, :], t2[rows, :], ALU.add)
                    thc = tpool.tile([128, CW], f32, tag=f"thc{ch}")
                    nc.scalar.activation(thc[:], c[:], AF.Tanh)
                    nc.vector.tensor_tensor(h[:], sog[:, 0:CW], thc[:], ALU.mult)

            for p in range(T_ + 1):
                encode_phase(p)

            # ---- transition to AR layout (per-chunk tiles) ----
            h0t, h1b, c0t, c1t = [], [], [], []
            for ch in range(CH):
                cs = slice(ch * CW, (ch + 1) * CW)
                a = spool.tile([65, CW], f32r, tag=f"h0t{ch}")
                nc.sync.dma_start(a[0:64, :], h_st[ch][0:64, :])
                nc.sync.dma_start(a[64:65, :], ones_d.ap()[0:1, cs].bitcast(f32r))
                h0t.append(a)
                b = spool.tile([65, CW], f32r, tag=f"h1b{ch}")
                nc.sync.dma_start(b[0:64, :], h_st[ch][64:128, :])
                nc.sync.dma_start(b[64:65, :], ones_d.ap()[0:1, cs].bitcast(f32r))
                h1b.append(b)
                c0 = spool.tile([64, CW], f32, tag=f"c0t{ch}")
                nc.sync.dma_start(c0[:], c_st[ch][0:64, :])
                c0t.append(c0)
                c1 = spool.tile([64, CW], f32, tag=f"c1t{ch}")
                nc.sync.dma_start(c1[:], c_st[ch][64:128, :])
                c1t.append(c1)

            def ar_mms(ch, wh0, wh1b):
                pq_if = ppool.tile([64, 2 * CW], f32, tag=f"pif{ch}")
                pq_og = ppool.tile([64, 2 * CW], f32, tag=f"pog{ch}")
                for (ps, col, g) in ((pq_if, 0, 0), (pq_if, 1, 1),
                                     (pq_og, 1, 3), (pq_og, 0, 2)):
                    dst = ps[:, col * CW:(col + 1) * CW]
                    nc.tensor.matmul(dst, wh0[:, g * 64:(g + 1) * 64],
                                     h0t[ch][:], start=True, stop=False)
                    nc.tensor.matmul(dst, wh1b[:, g * 64:(g + 1) * 64],
                                     h1b[ch][:], start=False, stop=True)
                return pq_if, pq_og

            def ar_tail(ch, pq_if, pq_og, ct, hout_t, hout_rows):
                sif = apool.tile([64, 2 * CW], f32, tag=f"sif{ch}")
                nc.scalar.activation(sif[:], pq_if[:], AF.Sigmoid)
                gt = tpool.tile([64, CW], f32, tag=f"gt{ch}")
                nc.scalar.activation(gt[:], pq_og[:, CW:], AF.Tanh)
                t2 = tpool.tile([64, CW], f32, tag=f"t2{ch}")
                nc.vector.tensor_tensor(t2[:], sif[:, CW:], ct[:], ALU.mult)
                so = tpool.tile([64, CW], f32, tag=f"so{ch}")
                nc.scalar.activation(so[:], pq_og[:, 0:CW], AF.Sigmoid)
                t1 = tpool.tile([64, CW], f32, tag=f"t1{ch}")
                nc.vector.tensor_tensor(t1[:], sif[:, 0:CW], gt[:], ALU.mult)
                nc.vector.tensor_tensor(ct[:], t1[:], t2[:], ALU.add)
                thc = tpool.tile([64, CW], f32, tag=f"thc{ch}")
                nc.scalar.activation(thc[:], ct[:], AF.Tanh)
                nc.vector.tensor_tensor(hout_t[hout_rows, :], so[:], thc[:], ALU.mult)

            for s in range(T_, T_ + STEPS_):
                last = s == T_ + STEPS_ - 1
                for ch in range(CH):
                    cs = slice(ch * CW, (ch + 1) * CW)
                    p1if, p1og = ar_mms(ch, w1h0, w1h1b)
                    ar_tail(ch, p1if, p1og, c1t[ch], h1b[ch], slice(0, 64))
                    pp = ppool.tile([4, CW], f32, tag=f"pif{ch}")
                    nc.tensor.matmul(pp[:], wfcb[:], h1b[ch][:], start=True, stop=True)
                    po = tpool.tile([4, CW], f32, tag=f"po{ch}")
                    nc.vector.tensor_copy(po[:], pp[:])
                    nc.sync.dma_start(out_d.ap()[s - T_, :, cs], po[:])
                    if not last:
                        p0if, p0og = ar_mms(ch, w0h0, w0h1b)
                        ar_tail(ch, p0if, p0og, c0t[ch], h0t[ch], slice(0, 64))

    nc.compile()
    return nc


def _prep_inputs(x, Wih0, Whh0, bih0, bhh0, Wih1, Whh1, bih1, bhh1, Wfc, bfc,
                 T_, STEPS_):
    """Host-side layout prep. Returns (shared_weight_arrays, per_core_xt)."""
    f = np.float32
    x = np.asarray(x, f)
    Wih0, Whh0 = np.asarray(Wih0, f), np.asarray(Whh0, f)
    Wih1, Whh1 = np.asarray(Wih1, f), np.asarray(Whh1, f)
    Wfc = np.asarray(Wfc, f)
    b0 = np.asarray(bih0, f) + np.asarray(bhh0, f)   # [4H]
    b1 = np.asarray(bih1, f) + np.asarray(bhh1, f)
    bfc = np.asarray(bfc, f)

    def gate(Wm, q):
        return Wm[q * H:(q + 1) * H]

    # encode block lhsT [128k, 4 gates x 128m]
    wencblk = np.zeros((128, 512), f)
    wencx = np.zeros((128, 512), f)
    for q in range(4):
        blk = np.zeros((128, 128), f)
        blk[0:64, 0:64] = gate(Whh0, q).T
        blk[0:64, 64:128] = gate(Wih1, q).T
        blk[64:128, 64:128] = gate(Whh1, q).T
        wencblk[:, q * 128:(q + 1) * 128] = blk
        wx = np.zeros((5, 128), f)
        wx[0:4, 0:64] = gate(Wih0, q).T
        wx[4, 0:64] = gate(b0[:, None], q)[:, 0]
        wx[4, 64:128] = gate(b1[:, None], q)[:, 0]
        wencx[0:5, q * 128:(q + 1) * 128] = wx

    # AR weights: lhsT col blocks = gates in order (i, f, o, g), M=64 each
    Wcomb = Wih0 @ Wfc          # [4H, 64]
    b0p = b0 + Wih0 @ bfc
    order = (0, 1, 2, 3)        # pytorch q for i, f, g, o is 0,1,2,3; cols i,f,o,g
    colq = (0, 1, 3, 2)
    w1h0 = np.zeros((65, 256), f)
    w1h1b = np.zeros((65, 256), f)
    w0h0 = np.zeros((65, 256), f)
    w0h1b = np.zeros((65, 256), f)
    for j, q in enumerate(colq):
        mc = slice(j * 64, (j + 1) * 64)
        w1h0[0:64, mc] = gate(Wih1, q).T
        w1h1b[0:64, mc] = gate(Whh1, q).T
        w1h1b[64, mc] = gate(b1[:, None], q)[:, 0]
        w0h0[0:64, mc] = gate(Whh0, q).T
        w0h1b[0:64, mc] = gate(Wcomb, q).T
        w0h1b[64, mc] = gate(b0p[:, None], q)[:, 0]
    wfcb = np.zeros((65, 4), f)
    wfcb[0:64] = Wfc.T
    wfcb[64] = bfc

    shared = dict(wencblk=wencblk, wencx=wencx, w1h0=w1h0, w1h1b=w1h1b,
                  w0h0=w0h0, w0h1b=w0h1b, wfcb=wfcb,
                  ones_row=np.ones((1, BC), f))

    # per-core x-tilde: [T+1, 5, BC]; row 4 = ones; step T duplicates x_{T-1}
    xts = []
    for c in range(NCORES):
        xs = x[c * BC:(c + 1) * BC, :T_, :]          # [BC, T_, D]
        xt = np.ones((T_ + 1, 5, BC), f)
        xt[:T_, 0:4, :] = np.transpose(xs, (1, 2, 0))
        xt[T_, 0:4, :] = xs[:, T_ - 1, :].T
        xts.append(xt)
    return shared, xts


def kernel(**inputs):
    return _run(T, STEPS, **inputs)


def _run(T_, STEPS_, x, Wih0, Whh0, bih0, bhh0, Wih1, Whh1, bih1, bhh1,
         Wfc, bfc):
    from concourse.bass_utils import run_bass_kernel_spmd

    key = (T_, STEPS_)
    if key not in _cache:
        _cache[key] = _build(T_, STEPS_)
    nc = _cache[key]

    shared, xts = _prep_inputs(x, Wih0, Whh0, bih0, bhh0, Wih1, Whh1,
                               bih1, bhh1, Wfc, bfc, T_, STEPS_)
    in_maps = [{**shared, "xt": xts[c]} for c in range(NCORES)]
    res = run_bass_kernel_spmd(nc, in_maps, core_ids=list(range(NCORES)),
                               trace=TRACE)
    LAST["exec_time_ns"] = res.exec_time_ns
    LAST["res"] = res
    out = np.empty((B, STEPS_, 4), np.float32)
    for c in range(NCORES):
        # res: [STEPS, 4, BC] -> [BC, STEPS, 4]
        out[c * BC:(c + 1) * BC] = np.transpose(res.results[c]["out"], (2, 0, 1))
    return out


# revision 20
# speedup vs baseline: 1.6151x; 1.0167x over previous
"""Trainium2 Bass kernel for nn_DigitalTwinSimulator (2-layer LSTM + AR rollout).

Strategy: pure data parallel across 8 NeuronCores (batch 4096 -> 512/core).
Per core, state is kept feature-on-partitions / batch-on-free-dim.

Encode phase (t = 0..128): the two LSTM layers run in lockstep (layer 1 lags
layer 0 by one step) so all gate matmuls merge into K=128 block matmuls and
all elementwise ops are full 128-partition instructions. Biases + x
contribution enter through a K=5 matmul against [x_t; 1] (ones row built
host-side).

AR phase (steps 128..187): pred feedback is algebraically folded:
  W0x @ pred = (W0x @ Wfc) @ h1 + W0x @ bfc
so the recurrence never materializes pred; the FC head runs off the critical
path purely for output. Layers run sequentially (inherent to AR); biases ride
in a 65th row of the h1 tile (ones row).

Batch is split into 2 chunks of 256 columns that pipeline against each other
to hide the serial dependency chain latency.
"""
import os
import sys

for _p in ("/opt/trn_rl_repo", "/root/.axon_site/_ro/trn_rl_repo"):
    if os.path.isdir(_p) and _p not in sys.path:
        sys.path.append(_p)

import numpy as np

B, T, D, H, STEPS = 4096, 128, 4, 64, 60
NCORES = 8
BC = B // NCORES          # 512 batch rows per core
CH = 2                    # batch chunks per core (pipelined)
CW = BC // CH             # 256 columns per chunk

_cache = {}
TRACE = False
LAST = {}


def _build(T_, STEPS_):
    import concourse.bass as bass
    import concourse.tile as tile
    from concourse import bacc, mybir

    f32 = mybir.dt.float32
    f32r = mybir.dt.float32r
    AF = mybir.ActivationFunctionType
    ALU = mybir.AluOpType

    nc = bacc.Bacc("TRN2", target_bir_lowering=False, debug=False,
                   num_devices=NCORES)

    xt_d = nc.dram_tensor("xt", (T_ + 1, 5, BC), f32, kind="ExternalInput")
    wencblk_d = nc.dram_tensor("wencblk", (128, 512), f32, kind="ExternalInput")
    wencx_d = nc.dram_tensor("wencx", (128, 512), f32, kind="ExternalInput")
    w1h0_d = nc.dram_tensor("w1h0", (65, 256), f32, kind="ExternalInput")
    w1h1b_d = nc.dram_tensor("w1h1b", (65, 256), f32, kind="ExternalInput")
    w0h0_d = nc.dram_tensor("w0h0", (65, 256), f32, kind="ExternalInput")
    w0h1b_d = nc.dram_tensor("w0h1b", (65, 256), f32, kind="ExternalInput")
    wfcb_d = nc.dram_tensor("wfcb", (65, 4), f32, kind="ExternalInput")
    ones_d = nc.dram_tensor("ones_row", (1, BC), f32, kind="ExternalInput")
    out_d = nc.dram_tensor("out", (STEPS_, 4, BC), f32, kind="ExternalOutput")

    with tile.TileContext(nc) as tc:
        with tc.tile_pool(name="const", bufs=1) as cpool, \
             tc.tile_pool(name="state", bufs=1) as spool, \
             tc.tile_pool(name="act", bufs=3) as apool, \
             tc.tile_pool(name="tmp", bufs=3) as tpool, \
             tc.tile_pool(name="psum", bufs=2, space="PSUM") as ppool:

            def dma_w(shape, src, tag, dt=f32r):
                t = cpool.tile(list(shape), dt, tag=tag)
                nc.sync.dma_start(t[:], src.ap().bitcast(dt))
                return t

            wencblk = dma_w((128, 512), wencblk_d, "wencblk")
            wencx = dma_w((128, 512), wencx_d, "wencx")
            w1h0 = dma_w((65, 256), w1h0_d, "w1h0")
            w1h1b = dma_w((65, 256), w1h1b_d, "w1h1b")
            w0h0 = dma_w((65, 256), w0h0_d, "w0h0")
            w0h1b = dma_w((65, 256), w0h1b_d, "w0h1b")
            wfcb = dma_w((65, 4), wfcb_d, "wfcb")

            # PE warm-up burst to flip the HAM clock gate before phase 0
            pwarm = ppool.tile([128, 256], f32, tag="pif0")
            for _w in range(40):
                nc.tensor.matmul(pwarm[:, (_w % 2) * 128:(_w % 2) * 128 + 128],
                                 wencblk[:, 0:128], wencblk[:, 0:128],
                                 start=True, stop=True)

            # per-chunk persistent state (separate tiles so the two batch
            # chunks share no tiles -> no false cross-chunk dependencies)
            zsrc = spool.tile([128, BC], f32, tag="zsrc")
            nc.vector.memset(zsrc[:], 0.0)
            h_st, c_st, xins = [], [], []
            for ch in range(CH):
                h = spool.tile([128, CW], f32r, tag=f"h_st{ch}")
                nc.vector.tensor_copy(h[:], zsrc[:, 0:CW])
                h_st.append(h)
                c = spool.tile([128, CW], f32, tag=f"c_st{ch}")
                nc.vector.memset(c[:], 0.0)
                c_st.append(c)
            # x staging: rows 5:128 stay zero => x matmul runs at K=128
            for _i in range(3):
                xt_t = spool.tile([128, BC], f32r, tag=f"xin{_i}")
                nc.vector.tensor_copy(xt_t[:], zsrc[:])
                xins.append(xt_t)

            def encode_phase(p):
                xin = xins[p % 3]
                nc.sync.dma_start(xin[0:5, :], xt_d.ap()[p].bitcast(f32r))
                for ch in range(CH):
                    cs = slice(ch * CW, (ch + 1) * CW)
                    h, c = h_st[ch], c_st[ch]
                    pif = ppool.tile([128, 2 * CW], f32, tag=f"pif{ch}")
                    pog = ppool.tile([128, 2 * CW], f32, tag=f"pog{ch}")
                    for (ps, col, q) in ((pif, 0, 0), (pif, 1, 1),
                                         (pog, 1, 2), (pog, 0, 3)):
                        dst = ps[:, col * CW:(col + 1) * CW]
                        nc.tensor.matmul(dst, wencblk[:, q * 128:(q + 1) * 128],
                                         h[:], start=True, stop=False)
                        nc.tensor.matmul(dst, wencx[:, q * 128:(q + 1) * 128],
                                         xin[:, cs], start=False, stop=True)
                    sif = apool.tile([128, 2 * CW], f32, tag=f"sif{ch}")
                    nc.scalar.activation(sif[:], pif[:], AF.Sigmoid)
                    sog = apool.tile([128, 2 * CW], f32, tag=f"sog{ch}")
                    nc.scalar.activation(sog[:, CW:], pog[:, CW:], AF.Tanh)
                    t2 = tpool.tile([128, CW], f32, tag=f"t2{ch}")
                    nc.vector.tensor_tensor(t2[:], sif[:, CW:], c[:], ALU.mult)
                    nc.scalar.activation(sog[:, 0:CW], pog[:, 0:CW], AF.Sigmoid)
                    t1 = tpool.tile([128, CW], f32, tag=f"t1{ch}")
                    nc.vector.tensor_tensor(t1[:], sif[:, 0:CW], sog[:, CW:], ALU.mult)
                    rows = slice(0, 64) if p == 0 else slice(0, 128)
                    nc.vector.tensor_tensor(c[rows, :], t1[rows, :], t2[rows, :], ALU.add)
                    thc = tpool.tile([128, CW], f32, tag=f"thc{ch}")
                    nc.scalar.activation(thc[:], c[:], AF.Tanh)
                    nc.vector.tensor_tensor(h[:], sog[:, 0:CW], thc[:], ALU.mult)

            for p in range(T_ + 1):
                encode_phase(p)

            # ---- transition to AR layout (per-chunk tiles) ----
            h0t, h1b, c0t, c1t = [], [], [], []
            for ch in range(CH):
                cs = slice(ch * CW, (ch + 1) * CW)
                a = spool.tile([65, CW], f32r, tag=f"h0t{ch}")
                nc.sync.dma_start(a[0:64, :], h_st[ch][0:64, :])
                nc.sync.dma_start(a[64:65, :], ones_d.ap()[0:1, cs].bitcast(f32r))
                h0t.append(a)
                b = spool.tile([65, CW], f32r, tag=f"h1b{ch}")
                nc.sync.dma_start(b[0:64, :], h_st[ch][64:128, :])
                nc.sync.dma_start(b[64:65, :], ones_d.ap()[0:1, cs].bitcast(f32r))
                h1b.append(b)
                c0 = spool.tile([64, CW], f32, tag=f"c0t{ch}")
                nc.sync.dma_start(c0[:], c_st[ch][0:64, :])
                c0t.append(c0)
                c1 = spool.tile([64, CW], f32, tag=f"c1t{ch}")
                nc.sync.dma_start(c1[:], c_st[ch][64:128, :])
                c1t.append(c1)

            def ar_mms(ch, wh0, wh1b):
                pq_if = ppool.tile([64, 2 * CW], f32, tag=f"pif{ch}")
                pq_og = ppool.tile([64, 2 * CW], f32, tag=f"pog{ch}")
                for (ps, col, g) in ((pq_if, 0, 0), (pq_if, 1, 1),
                                     (pq_og, 1, 3), (pq_og, 0, 2)):
                    dst = ps[:, col * CW:(col + 1) * CW]
                    nc.tensor.matmul(dst, wh0[:, g * 64:(g + 1) * 64],
                                     h0t[ch][:], start=True, stop=False)
                    nc.tensor.matmul(dst, wh1b[:, g * 64:(g + 1) * 64],
                                     h1b[ch][:], start=False, stop=True)
                return pq_if, pq_og

            def ar_tail(ch, pq_if, pq_og, ct, hout_t, hout_rows):
                sif = apool.tile([64, 2 * CW], f32, tag=f"sif{ch}")
                nc.scalar.activation(sif[:], pq_if[:], AF.Sigmoid)
                gt = tpool.tile([64, CW], f32, tag=f"gt{ch}")
                nc.scalar.activation(gt[:], pq_og[:, CW:], AF.Tanh)
                t2 = tpool.tile([64, CW], f32, tag=f"t2{ch}")
                nc.vector.tensor_tensor(t2[:], sif[:, CW:], ct[:], ALU.mult)
                so = tpool.tile([64, CW], f32, tag=f"so{ch}")
                nc.scalar.activation(so[:], pq_og[:, 0:CW], AF.Sigmoid)
                t1 = tpool.tile([64, CW], f32, tag=f"t1{ch}")
                nc.vector.tensor_tensor(t1[:], sif[:, 0:CW], gt[:], ALU.mult)
                nc.vector.tensor_tensor(ct[:], t1[:], t2[:], ALU.add)
                thc = tpool.tile([64, CW], f32, tag=f"thc{ch}")
                nc.scalar.activation(thc[:], ct[:], AF.Tanh)
                nc.vector.tensor_tensor(hout_t[hout_rows, :], so[:], thc[:], ALU.mult)

            for s in range(T_, T_ + STEPS_):
                last = s == T_ + STEPS_ - 1
                for ch in range(CH):
                    cs = slice(ch * CW, (ch + 1) * CW)
                    p1if, p1og = ar_mms(ch, w1h0, w1h1b)
                    ar_tail(ch, p1if, p1og, c1t[ch], h1b[ch], slice(0, 64))
                    # pred reuses a dead region of p1if (already consumed)
                    pp = p1if[0:4, 0:CW]
                    nc.tensor.matmul(pp, wfcb[:], h1b[ch][:], start=True, stop=True)
                    po = tpool.tile([4, CW], f32, tag=f"po{ch}")
                    nc.vector.tensor_copy(po[:], pp)
                    nc.sync.dma_start(out_d.ap()[s - T_, :, cs], po[:])
                    if not last:
                        p0if, p0og = ar_mms(ch, w0h0, w0h1b)
                        ar_tail(ch, p0if, p0og, c0t[ch], h0t[ch], slice(0, 64))

    nc.compile()
    return nc


def _prep_inputs(x, Wih0, Whh0, bih0, bhh0, Wih1, Whh1, bih1, bhh1, Wfc, bfc,
                 T_, STEPS_):
    """Host-side layout prep. Returns (shared_weight_arrays, per_core_xt)."""
    f = np.float32
    x = np.asarray(x, f)
    Wih0, Whh0 = np.asarray(Wih0, f), np.asarray(Whh0, f)
    Wih1, Whh1 = np.asarray(Wih1, f), np.asarray(Whh1, f)
    Wfc = np.asarray(Wfc, f)
    b0 = np.asarray(bih0, f) + np.asarray(bhh0, f)   # [4H]
    b1 = np.asarray(bih1, f) + np.asarray(bhh1, f)
    bfc = np.asarray(bfc, f)

    def gate(Wm, q):
        return Wm[q * H:(q + 1) * H]

    # encode block lhsT [128k, 4 gates x 128m]
    wencblk = np.zeros((128, 512), f)
    wencx = np.zeros((128, 512), f)
    for q in range(4):
        blk = np.zeros((128, 128), f)
        blk[0:64, 0:64] = gate(Whh0, q).T
        blk[0:64, 64:128] = gate(Wih1, q).T
        blk[64:128, 64:128] = gate(Whh1, q).T
        wencblk[:, q * 128:(q + 1) * 128] = blk
        wx = np.zeros((5, 128), f)
        wx[0:4, 0:64] = gate(Wih0, q).T
        wx[4, 0:64] = gate(b0[:, None], q)[:, 0]
        wx[4, 64:128] = gate(b1[:, None], q)[:, 0]
        wencx[0:5, q * 128:(q + 1) * 128] = wx

    # AR weights: lhsT col blocks = gates in order (i, f, o, g), M=64 each
    Wcomb = Wih0 @ Wfc          # [4H, 64]
    b0p = b0 + Wih0 @ bfc
    order = (0, 1, 2, 3)        # pytorch q for i, f, g, o is 0,1,2,3; cols i,f,o,g
    colq = (0, 1, 3, 2)
    w1h0 = np.zeros((65, 256), f)
    w1h1b = np.zeros((65, 256), f)
    w0h0 = np.zeros((65, 256), f)
    w0h1b = np.zeros((65, 256), f)
    for j, q in enumerate(colq):
        mc = slice(j * 64, (j + 1) * 64)
        w1h0[0:64, mc] = gate(Wih1, q).T
        w1h1b[0:64, mc] = gate(Whh1, q).T
        w1h1b[64, mc] = gate(b1[:, None], q)[:, 0]
        w0h0[0:64, mc] = gate(Whh0, q).T
        w0h1b[0:64, mc] = gate(Wcomb, q).T
        w0h1b[64, mc] = gate(b0p[:, None], q)[:, 0]
    wfcb = np.zeros((65, 4), f)
    wfcb[0:64] = Wfc.T
    wfcb[64] = bfc

    shared = dict(wencblk=wencblk, wencx=wencx, w1h0=w1h0, w1h1b=w1h1b,
                  w0h0=w0h0, w0h1b=w0h1b, wfcb=wfcb,
                  ones_row=np.ones((1, BC), f))

    # per-core x-tilde: [T+1, 5, BC]; row 4 = ones; step T duplicates x_{T-1}
    xts = []
    for c in range(NCORES):
        xs = x[c * BC:(c + 1) * BC, :T_, :]          # [BC, T_, D]
        xt = np.ones((T_ + 1, 5, BC), f)
        xt[:T_, 0:4, :] = np.transpose(xs, (1, 2, 0))
        xt[T_, 0:4, :] = xs[:, T_ - 1, :].T
        xts.append(xt)
    return shared, xts


def kernel(**inputs):
    return _run(T, STEPS, **inputs)


def _run(T_, STEPS_, x, Wih0, Whh0, bih0, bhh0, Wih1, Whh1, bih1, bhh1,
         Wfc, bfc):
    from concourse.bass_utils import run_bass_kernel_spmd

    key = (T_, STEPS_)
    if key not in _cache:
        _cache[key] = _build(T_, STEPS_)
    nc = _cache[key]

    shared, xts = _prep_inputs(x, Wih0, Whh0, bih0, bhh0, Wih1, Whh1,
                               bih1, bhh1, Wfc, bfc, T_, STEPS_)
    in_maps = [{**shared, "xt": xts[c]} for c in range(NCORES)]
    res = run_bass_kernel_spmd(nc, in_maps, core_ids=list(range(NCORES)),
                               trace=TRACE)
    LAST["exec_time_ns"] = res.exec_time_ns
    LAST["res"] = res
    out = np.empty((B, STEPS_, 4), np.float32)
    for c in range(NCORES):
        # res: [STEPS, 4, BC] -> [BC, STEPS, 4]
        out[c * BC:(c + 1) * BC] = np.transpose(res.results[c]["out"], (2, 0, 1))
    return out
